# revision 19
# baseline (speedup 1.0000x reference)
"""Cross-attention kernel for Trainium2, sharded over 8 NeuronCores.

v2: query rows sharded (1024/core) AND the K/V projections sharded by key
group (1024 keys/core) instead of replicated.  Each core projects only its
own key group (1/8 of the old PE work), then a packed shared-output
AllGather distributes the projected {K^T, V} slabs (2MB fp16 in, 16MB out)
through DRAM.  Attention processes key groups in rotated order starting
with the core's OWN group (read straight from SBUF), so the collective's
~70us mesh latency hides behind Q-projection + own-group attention.

Structure:
  - Host pre-marshals inputs (fp32->fp16 cast + transpose into
    contraction-on-partition layouts).  Per core: own q rows + own k/v keys.
  - Device: project own K^T/V into one packed slab [P, 2, 8, 512] fp16 ->
    DMA to DRAM -> AllGather into a Shared 16MB buffer.  Remote groups are
    DMA-streamed from ccout[(pid+j)%8] (dynamic index on the core id).
  - Attention is one unified software-pipelined slot stream over all
    (group, row-block) pairs: slot s = [T(s-2), QK(s), PV(s-3)], flash
    online-softmax state ping-pongs by group parity.  Q-projection rh=1
    chunks fill the PE during the first slots' softmax latency.
  - PSUM budget 6 banks: 3 proj/PV + 2 scores + 1 A^T.

Algebraic simplifications (as v1):
  - bk dropped (softmax cancels it), bv folded into the epilogue,
  - softmax 1/l and the post-softmax 1/sqrt(dk) fold into one per-row
    multiply at the end.
"""
import sys

sys.path.insert(0, "/opt/trn_rl_repo")

import numpy as np  # noqa: E402
import concourse.bass as bass  # noqa: E402
import concourse.tile as tile  # noqa: E402
from concourse import mybir  # noqa: E402
from concourse import bass_utils  # noqa: E402
from contextlib import ExitStack  # noqa: E402

F16 = mybir.dt.float16
F32 = mybir.dt.float32
AF = mybir.ActivationFunctionType
AX = mybir.AxisListType
ALU = mybir.AluOpType

P = 128
D = 1024             # input dim
ND = D // P          # 8 d-chunks
C = 512              # dim_k
NCC = C // P         # 4 c-chunks
VD = 512             # dim_v
KEYS = 8192
GK = 1024            # keys per group
NG = KEYS // GK      # 8 groups
RL = 1024            # query rows per core
NB = RL // P         # 8 row blocks
NCORES = 8
NORM = float(1.0 / np.sqrt(np.float32(C)))

_ws_counter = [0]


def _split_multi_waits(nc):
    """This container's walrus accepts only ONE sync-wait per instruction.
    Move extra waits onto preceding same-engine EventSemaphore insts."""
    for f in nc.m.functions:
        for bb in f.blocks:
            il = bb.instructions
            if not any(
                inst.sync_info is not None and len(inst.sync_info.on_wait or ()) > 1
                for inst in il
            ):
                continue
            new = []
            for inst in il:
                si = inst.sync_info
                if si is not None and len(si.on_wait or ()) > 1:
                    waits = list(si.on_wait)
                    for w in waits[:-1]:
                        _ws_counter[0] += 1
                        new.append(
                            mybir.InstEventSemaphore(
                                name=f"I-ws{_ws_counter[0]}",
                                engine=inst.engine,
                                ins=[],
                                outs=[],
                                sync_info=mybir.SyncInfo(on_wait=[w], on_update=[]),
                            )
                        )
                    del si.on_wait[:-1]
                new.append(inst)
            bb.instructions = new


def _emit(nc, tc, aps):
    qT_r = aps["qT"]
    kTo_r = aps["kTo"]
    vTo_r = aps["vTo"]
    out_ap = aps["out"]
    ccout = aps["ccout"]

    with ExitStack() as top:
        const = top.enter_context(tc.tile_pool(name="const", bufs=1))
        kvown = top.enter_context(tc.tile_pool(name="kvown", bufs=1))
        kvin = top.enter_context(tc.tile_pool(name="kvin", bufs=3))
        apool = top.enter_context(tc.tile_pool(name="apool", bufs=4))
        atp = top.enter_context(tc.tile_pool(name="atp", bufs=6))
        stat = top.enter_context(tc.tile_pool(name="stat", bufs=6))
        outp = top.enter_context(tc.tile_pool(name="outp", bufs=2))
        dram = top.enter_context(tc.tile_pool(name="dram", bufs=1, space="DRAM"))
        pp = top.enter_context(tc.tile_pool(name="pp", bufs=3, space="PSUM"))
        psS = top.enter_context(tc.tile_pool(name="psS", bufs=2, space="PSUM"))
        pat = top.enter_context(tc.tile_pool(name="pat", bufs=1, space="PSUM"))

        # ---- persistent operands ----
        # sync queue order: wk + own kv slabs first (they gate the proj),
        # then wq + q rows (gate Q-proj), then the remote slab loads.
        wq = const.tile([P, ND, C], F16, tag="wq")
        wk = const.tile([P, ND, C], F16, tag="wk")
        wv = const.tile([P, ND, C], F16, tag="wv")
        nc.sync.dma_start(wk[:], aps["wkT"][:])
        kto = kvown.tile([P, ND, GK], F16, tag="kto")
        vto = kvown.tile([P, ND, GK], F16, tag="vto")
        nc.sync.dma_start(kto[:], kTo_r[:])
        nc.sync.dma_start(wv[:], aps["wvT"][:])
        nc.sync.dma_start(vto[:], vTo_r[:])
        nc.sync.dma_start(wq[:], aps["wqT"][:])
        qin = kvown.tile([P, 2, ND, 512], F16, tag="qin")
        nc.sync.dma_start(qin[:, 0], qT_r[:, 0])
        nc.sync.dma_start(qin[:, 1], qT_r[:, 1])
        bvrow = const.tile([1, VD], F32, tag="bvrow")
        nc.scalar.dma_start(bvrow[:], aps["bv"][None, :])
        bqT = const.tile([P, NCC], F32, tag="bqT")
        nc.scalar.dma_start(bqT[:], aps["bqT"][:])
        ident = const.tile([P, P], F16, tag="ident")
        nc.scalar.dma_start(ident[:], aps["ident"][:])
        ones1 = const.tile([1, P], F32, tag="ones1")
        nc.vector.memset(ones1[:], 1.0)

        # ---- own-group K^T / V projection into the packed slab ----
        # slab[:, 0, 2*ci+h, :] = K^T chunk (ci, h)  ([c-part, keys])
        # slab[:, 1, kc, :]     = V chunk kc         ([key-part, v])
        # Each chunk is streamed to the DRAM collective-input buffer as soon
        # as its PSUM eviction lands, so the AllGather fires right after the
        # last chunk instead of waiting for a bulk 2MB copy.
        slab_own = kvown.tile([P, 2, ND, 512], F16, tag="own")
        ccin = dram.tile([P, 2, ND, 512], F16)
        for ci in range(NCC):
            for h in range(2):
                ps = pp.tile([P, 512], F32, tag="pp", name="psk")
                for d in range(ND):
                    nc.tensor.matmul(
                        ps[:],
                        wk[:, d, ci * P:(ci + 1) * P],
                        kto[:, d, h * 512:(h + 1) * 512],
                        start=(d == 0),
                        stop=(d == ND - 1),
                    )
                nc.scalar.copy(slab_own[:, 0, 2 * ci + h, :], ps[:])
                nc.gpsimd.dma_start(
                    ccin[:, 0, 2 * ci + h, :], slab_own[:, 0, 2 * ci + h, :]
                )
        for kc in range(ND):
            ps = pp.tile([P, 512], F32, tag="pp", name="psv")
            for d in range(ND):
                nc.tensor.matmul(
                    ps[:],
                    vto[:, d, kc * P:(kc + 1) * P],
                    wv[:, d, :],
                    start=(d == 0),
                    stop=(d == ND - 1),
                )
            nc.scalar.copy(slab_own[:, 1, kc, :], ps[:])
            nc.gpsimd.dma_start(ccin[:, 1, kc, :], slab_own[:, 1, kc, :])

        # ---- AllGather all groups into shared DRAM ----
        nc.gpsimd.collective_compute(
            "AllGather",
            ALU.bypass,
            replica_groups=[list(range(NCORES))],
            ins=[ccin.opt()],
            outs=[ccout.ap()],
        )

        pid = nc.sync.partition_id()
        pid_sc = nc.scalar.partition_id()
        slabs = {0: slab_own}

        pid_gp = nc.gpsimd.partition_id()
        _engs = [(nc.sync, pid), (nc.scalar, pid_sc), (nc.gpsimd, pid_gp)]

        def load_slab(j):
            # Split each slab load across the three DMA queues, and rotate
            # which queue carries the K^T half (it gates QK of the group):
            # this way the first post-collective K^T loads don't serialize
            # behind each other on one queue.
            ek, pk = _engs[(j - 1) % 3]
            ev0, pv0 = _engs[j % 3]
            ev1, pv1 = _engs[(j + 1) % 3]
            t = kvin.tile([P, 2, ND, 512], F16, tag="slab", name=f"slab{j}")
            ek.dma_start(t[:, 0], ccout[(pk + j) % NG][:, 0])
            ev0.dma_start(t[:, 1, 0:4], ccout[(pv0 + j) % NG][:, 1, 0:4])
            ev1.dma_start(t[:, 1, 4:8], ccout[(pv1 + j) % NG][:, 1, 4:8])
            slabs[j] = t

        # eager: queue the first three remote slabs on three different DMA
        # queues; they all gate on the collective's completion semaphore.
        load_slab(1)
        load_slab(2)
        load_slab(3)

        # ---- Q^T projection (rh=0 now; rh=1 chunks fill early slots) ----
        QT = const.tile([P, NCC, RL], F16, tag="QT")

        def q_chunk(rh, ci):
            ps = pp.tile([P, 512], F32, tag="pp", name="psq")
            for d in range(ND):
                nc.tensor.matmul(
                    ps[:],
                    wq[:, d, ci * P:(ci + 1) * P],
                    qin[:, rh, d, :],
                    start=(d == 0),
                    stop=(d == ND - 1),
                )
            nc.scalar.activation(
                QT[:, ci, rh * 512:(rh + 1) * 512],
                ps[:],
                AF.Identity,
                bias=bqT[:, ci:ci + 1],
                scale=1.0,
            )

        for ci in range(NCC):
            q_chunk(0, ci)

        # bvN[p, v] = NORM * bv[v] broadcast along partitions (rank-1 matmul)
        bvN = const.tile([P, VD], F32, tag="bvN")
        psb0 = pp.tile([P, VD], F32, tag="pp")
        nc.tensor.matmul(psb0[:], ones1[:], bvrow[:], start=True, stop=True)
        nc.scalar.activation(bvN[:], psb0[:], AF.Copy, bias=0.0, scale=NORM)

        # flash state, ping-pong by group-slot parity
        m_st = [const.tile([P, NB], F32, tag=f"m{i}", name=f"m{i}") for i in range(2)]
        rs_st = [const.tile([P, NB], F32, tag=f"rs{i}", name=f"rs{i}") for i in range(2)]
        O_st = [
            const.tile([P, NB, VD], F32, tag=f"O{i}", name=f"O{i}") for i in range(2)
        ]

        a_h = {}
        at_h = {}
        f_h = {}

        def qk(j, b):
            KT = slabs[j]
            m_in, m_out = m_st[j % 2], m_st[1 - j % 2]
            rs_in, rs_out = rs_st[j % 2], rs_st[1 - j % 2]
            S0 = psS.tile([P, 512], F32, tag="S0", name=f"S0_{j}_{b}")
            S1 = psS.tile([P, 512], F32, tag="S1", name=f"S1_{j}_{b}")
            for S, h in ((S0, 0), (S1, 1)):
                for ci in range(NCC):
                    nc.tensor.matmul(
                        S[:],
                        QT[:, ci, b * P:(b + 1) * P],
                        KT[:, 0, 2 * ci + h, :],
                        start=(ci == 0),
                        stop=(ci == NCC - 1),
                    )
            gm0 = stat.tile([P, 1], F32, tag="gm0", name="gm0")
            gm1 = stat.tile([P, 1], F32, tag="gm1", name="gm1")
            nc.vector.reduce_max(gm0[:], S0[:], axis=AX.X)
            nc.vector.reduce_max(gm1[:], S1[:], axis=AX.X)
            mb = m_out[:, b:b + 1]
            if j == 0:
                nc.vector.tensor_tensor(mb, gm0[:], gm1[:], op=ALU.max)
            else:
                g01 = stat.tile([P, 1], F32, tag="g01", name="g01")
                nc.vector.tensor_tensor(g01[:], gm0[:], gm1[:], op=ALU.max)
                nc.vector.tensor_tensor(mb, m_in[:, b:b + 1], g01[:], op=ALU.max)
            negm = stat.tile([P, 1], F32, tag="negm", name="negm")
            nc.vector.tensor_scalar(negm[:], mb, -1.0, None, op0=ALU.mult)
            if j > 0:
                f = stat.tile([P, 1], F32, tag="f", name="f")
                nc.scalar.activation(
                    f[:], m_in[:, b:b + 1], AF.Exp, bias=negm[:], scale=1.0
                )
                f_h[(j, b)] = f
            A = apool.tile([P, GK], F16, tag="A", name=f"A_{j}_{b}")
            ps0 = stat.tile([P, 1], F32, tag="ps0", name="ps0")
            ps1 = stat.tile([P, 1], F32, tag="ps1", name="ps1")
            nc.scalar.activation(
                A[:, 0:512], S0[:], AF.Exp, bias=negm[:], scale=1.0,
                accum_out=ps0[:],
            )
            nc.scalar.activation(
                A[:, 512:1024], S1[:], AF.Exp, bias=negm[:], scale=1.0,
                accum_out=ps1[:],
            )
            rb_in, rb_out = rs_in[:, b:b + 1], rs_out[:, b:b + 1]
            if j == 0:
                nc.vector.tensor_tensor(rb_out, ps0[:], ps1[:], op=ALU.add)
            else:
                pss = stat.tile([P, 1], F32, tag="pss", name="pss")
                nc.vector.tensor_tensor(pss[:], ps0[:], ps1[:], op=ALU.add)
                nc.vector.scalar_tensor_tensor(
                    rb_out, rb_in, f_h[(j, b)][:], pss[:], op0=ALU.mult, op1=ALU.add
                )
            a_h[(j, b)] = A

        def tr(j, b):
            # A^T for all 8 key chunks, PE transpose-mode -> one PSUM bank
            A = a_h.pop((j, b))
            tp = pat.tile([P, ND, P], F16, tag="tp", name=f"tp_{j}_{b}")
            for kc in range(ND):
                nc.tensor.transpose(
                    tp[:, kc, :], A[:, kc * P:(kc + 1) * P], ident[:]
                )
            AT = atp.tile([P, ND, P], F16, tag="AT", name=f"AT_{j}_{b}")
            nc.scalar.copy(AT[:], tp[:])
            at_h[(j, b)] = AT

        def pv(j, b):
            Vg = slabs[j]
            O_in, O_out = O_st[j % 2], O_st[1 - j % 2]
            po = pp.tile([P, VD], F32, tag="pp", name=f"po_{j}_{b}")
            AT = at_h.pop((j, b))
            for kc in range(ND):
                nc.tensor.matmul(
                    po[:],
                    AT[:, kc, :],
                    Vg[:, 1, kc, :],
                    start=(kc == 0),
                    stop=(kc == ND - 1),
                )
            ob_out, ob_in = O_out[:, b, :], O_in[:, b, :]
            if j == 0:
                nc.vector.tensor_copy(ob_out, po[:])
            else:
                nc.vector.scalar_tensor_tensor(
                    ob_out, ob_in, f_h.pop((j, b))[:], po[:],
                    op0=ALU.mult, op1=ALU.add,
                )

        def ep(j, b):
            # epilogue: out = O * (NORM / rs) + NORM * bv
            rs_out = rs_st[1 - j % 2]
            O_out = O_st[1 - j % 2]
            rinv = stat.tile([P, 1], F32, tag="rinv", name="rinv")
            nc.vector.reciprocal(rinv[:], rs_out[:, b:b + 1])
            rn = stat.tile([P, 1], F32, tag="rn", name="rn")
            nc.vector.tensor_scalar(rn[:], rinv[:], NORM, None, op0=ALU.mult)
            of = outp.tile([P, VD], F32, tag="of", name=f"of{b}")
            nc.vector.scalar_tensor_tensor(
                of[:], O_out[:, b, :], rn[:], bvN[:], op0=ALU.mult, op1=ALU.add
            )
            nc.scalar.dma_start(out_ap[b * P:(b + 1) * P, :], of[:])

        def pv_ep(j, b):
            pv(j, b)
            if j == NG - 1:
                ep(j, b)

        # early-slot PE filler: the rh=1 Q-proj chunks (needed from b=4 on)
        fillers = [
            lambda: (q_chunk(1, 0), q_chunk(1, 1)),
            lambda: q_chunk(1, 2),
            lambda: q_chunk(1, 3),
        ]

        NS = NG * NB
        for s in range(NS + 3):
            if 2 <= s < NS + 2:
                tr(*divmod(s - 2, NB))
            if s < NS:
                j, b = divmod(s, NB)
                qk(j, b)
                if s < 3:
                    fillers[s]()
                if b == 3 and j >= 1 and j + 3 < NG:
                    load_slab(j + 3)
            if s >= 3:
                pv_ep(*divmod(s - 3, NB))


_cached = {}


def _build():
    if "nc" in _cached:
        return _cached["nc"]
    nc = bass.Bass("TRN2", target_bir_lowering=False, debug=False,
                   num_devices=NCORES)
    ccout = nc.dram_tensor(
        "ccout", [NG, P, 2, ND, 512], F16, kind="Internal", addr_space="Shared"
    )
    aps = {
        "qT": nc.dram_tensor("qT", [P, 2, ND, 512], F16, kind="ExternalInput").ap(),
        "kTo": nc.dram_tensor("kTo", [P, ND, GK], F16, kind="ExternalInput").ap(),
        "vTo": nc.dram_tensor("vTo", [P, ND, GK], F16, kind="ExternalInput").ap(),
        "wqT": nc.dram_tensor("wqT", [P, ND, C], F16, kind="ExternalInput").ap(),
        "wkT": nc.dram_tensor("wkT", [P, ND, C], F16, kind="ExternalInput").ap(),
        "wvT": nc.dram_tensor("wvT", [P, ND, C], F16, kind="ExternalInput").ap(),
        "bqT": nc.dram_tensor("bqT", [P, NCC], F32, kind="ExternalInput").ap(),
        "bv": nc.dram_tensor("bv", [VD], F32, kind="ExternalInput").ap(),
        "ident": nc.dram_tensor("ident", [P, P], F16, kind="ExternalInput").ap(),
        "out": nc.dram_tensor("out", [RL, VD], F32, kind="ExternalOutput").ap(),
        "ccout": ccout,
    }
    with tile.TileContext(nc) as tc:
        _emit(nc, tc, aps)
    _split_multi_waits(nc)
    _cached["nc"] = nc
    return nc


def kernel(q, k, v, Wq, bq, Wk, bk, Wv, bv, _trace=False, _tmpdir=None):
    del bk  # provably cancels inside the softmax
    nc = _build()

    def f16(a):
        return np.asarray(a, dtype=np.float32).astype(np.float16)

    def kv_prep(a):
        # [keys, d] -> [p, group, d-chunk, key-in-group]
        return f16(a).reshape(NG, GK, ND, P).transpose(3, 0, 2, 1)

    def w_prep(a):
        # [c, d] -> [p, d-chunk, c]
        return np.ascontiguousarray(f16(a).reshape(C, ND, P).transpose(2, 1, 0))

    q16 = f16(q)
    kT = kv_prep(k)
    vT = kv_prep(v)
    base = {
        "wqT": w_prep(Wq), "wkT": w_prep(Wk), "wvT": w_prep(Wv),
        "bqT": np.ascontiguousarray(
            np.asarray(bq, dtype=np.float32).reshape(NCC, P).T
        ),
        "bv": np.ascontiguousarray(np.asarray(bv, dtype=np.float32)),
        "ident": np.eye(P, dtype=np.float16),
    }
    in_maps = [
        dict(
            base,
            qT=np.ascontiguousarray(
                q16[c * RL:(c + 1) * RL].reshape(2, 512, ND, P).transpose(3, 0, 2, 1)
            ),
            kTo=np.ascontiguousarray(kT[:, c]),
            vTo=np.ascontiguousarray(vT[:, c]),
        )
        for c in range(NCORES)
    ]
    res = bass_utils.run_bass_kernel_spmd(
        nc, in_maps, core_ids=list(range(NCORES)), trace=_trace, tmpdir=_tmpdir
    )
    out = np.concatenate([res.results[c]["out"] for c in range(NCORES)], axis=0)
    if _trace:
        kernel.last_results = res
    return out


# revision 21
# speedup vs baseline: 1.0352x; 1.0352x over previous
"""Cross-attention kernel for Trainium2, sharded over 8 NeuronCores.

v2: query rows sharded (1024/core) AND the K/V projections sharded by key
group (1024 keys/core) instead of replicated.  Each core projects only its
own key group (1/8 of the old PE work), then a packed shared-output
AllGather distributes the projected {K^T, V} slabs (2MB fp16 in, 16MB out)
through DRAM.  Attention processes key groups in rotated order starting
with the core's OWN group (read straight from SBUF), so the collective's
~70us mesh latency hides behind Q-projection + own-group attention.

Structure:
  - Host pre-marshals inputs (fp32->fp16 cast + transpose into
    contraction-on-partition layouts).  Per core: own q rows + own k/v keys.
  - Device: project own K^T/V into one packed slab [P, 2, 8, 512] fp16 ->
    DMA to DRAM -> AllGather into a Shared 16MB buffer.  Remote groups are
    DMA-streamed from ccout[(pid+j)%8] (dynamic index on the core id).
  - Attention is one unified software-pipelined slot stream over all
    (group, row-block) pairs: slot s = [T(s-2), QK(s), PV(s-3)], flash
    online-softmax state ping-pongs by group parity.  Q-projection rh=1
    chunks fill the PE during the first slots' softmax latency.
  - Projection chunks stream to the collective input buffer as they land;
    remote slab loads are split across the sync/scalar/gpsimd DMA queues
    with the QK-gating K^T half rotated onto its own queue.
  - PSUM budget 8 banks: 3 proj/PV + 2x2 scores (double-buffered) + 1 A^T.

Measured on the 8-core axon TRN2 pod: 430.0us, rel err 1.39e-2
(v1 replicated-KV baseline: 528us).  NOTE: any NEFF containing a
collective runs the whole core ~18% below the clock of a collective-free
NEFF (measured 216ns vs 259ns per [128,512] fp16 matmul) -- the sharded
projection still wins by ~100us despite that penalty.

Algebraic simplifications (as v1):
  - bk dropped (softmax cancels it), bv folded into the epilogue,
  - softmax 1/l and the post-softmax 1/sqrt(dk) fold into one per-row
    multiply at the end.
"""
import sys

sys.path.insert(0, "/opt/trn_rl_repo")

import numpy as np  # noqa: E402
import concourse.bass as bass  # noqa: E402
import concourse.tile as tile  # noqa: E402
from concourse import mybir  # noqa: E402
from concourse import bass_utils  # noqa: E402
from contextlib import ExitStack  # noqa: E402

F16 = mybir.dt.float16
F32 = mybir.dt.float32
AF = mybir.ActivationFunctionType
AX = mybir.AxisListType
ALU = mybir.AluOpType

P = 128
D = 1024             # input dim
ND = D // P          # 8 d-chunks
C = 512              # dim_k
NCC = C // P         # 4 c-chunks
VD = 512             # dim_v
KEYS = 8192
GK = 1024            # keys per group
NG = KEYS // GK      # 8 groups
RL = 1024            # query rows per core
NB = RL // P         # 8 row blocks
NCORES = 8
NORM = float(1.0 / np.sqrt(np.float32(C)))

_ws_counter = [0]


def _split_multi_waits(nc):
    """This container's walrus accepts only ONE sync-wait per instruction.
    Move extra waits onto preceding same-engine EventSemaphore insts."""
    for f in nc.m.functions:
        for bb in f.blocks:
            il = bb.instructions
            if not any(
                inst.sync_info is not None and len(inst.sync_info.on_wait or ()) > 1
                for inst in il
            ):
                continue
            new = []
            for inst in il:
                si = inst.sync_info
                if si is not None and len(si.on_wait or ()) > 1:
                    waits = list(si.on_wait)
                    for w in waits[:-1]:
                        _ws_counter[0] += 1
                        new.append(
                            mybir.InstEventSemaphore(
                                name=f"I-ws{_ws_counter[0]}",
                                engine=inst.engine,
                                ins=[],
                                outs=[],
                                sync_info=mybir.SyncInfo(on_wait=[w], on_update=[]),
                            )
                        )
                    del si.on_wait[:-1]
                new.append(inst)
            bb.instructions = new


def _emit(nc, tc, aps):
    qT_r = aps["qT"]
    kTo_r = aps["kTo"]
    vTo_r = aps["vTo"]
    out_ap = aps["out"]
    ccout = aps["ccout"]

    with ExitStack() as top:
        const = top.enter_context(tc.tile_pool(name="const", bufs=1))
        kvown = top.enter_context(tc.tile_pool(name="kvown", bufs=1))
        kvin = top.enter_context(tc.tile_pool(name="kvin", bufs=3))
        apool = top.enter_context(tc.tile_pool(name="apool", bufs=4))
        atp = top.enter_context(tc.tile_pool(name="atp", bufs=6))
        stat = top.enter_context(tc.tile_pool(name="stat", bufs=6))
        outp = top.enter_context(tc.tile_pool(name="outp", bufs=2))
        dram = top.enter_context(tc.tile_pool(name="dram", bufs=1, space="DRAM"))
        pp = top.enter_context(tc.tile_pool(name="pp", bufs=3, space="PSUM"))
        psS = top.enter_context(tc.tile_pool(name="psS", bufs=2, space="PSUM"))
        pat = top.enter_context(tc.tile_pool(name="pat", bufs=1, space="PSUM"))

        # ---- persistent operands ----
        # sync queue order: wk + own kv slabs first (they gate the proj),
        # then wq + q rows (gate Q-proj), then the remote slab loads.
        wq = const.tile([P, ND, C], F16, tag="wq")
        wk = const.tile([P, ND, C], F16, tag="wk")
        wv = const.tile([P, ND, C], F16, tag="wv")
        # wk + kto gate the K projection (and so the collective): spread the
        # first 3MB across all three DMA queues so the PE starts ~4us sooner.
        nc.sync.dma_start(wk[:], aps["wkT"][:])
        kto = kvown.tile([P, ND, GK], F16, tag="kto")
        vto = kvown.tile([P, ND, GK], F16, tag="vto")
        nc.scalar.dma_start(kto[:, 0:4], kTo_r[:, 0:4])
        nc.gpsimd.dma_start(kto[:, 4:8], kTo_r[:, 4:8])
        nc.sync.dma_start(wv[:], aps["wvT"][:])
        nc.scalar.dma_start(vto[:, 0:4], vTo_r[:, 0:4])
        nc.gpsimd.dma_start(vto[:, 4:8], vTo_r[:, 4:8])
        nc.sync.dma_start(wq[:], aps["wqT"][:])
        qin = kvown.tile([P, 2, ND, 512], F16, tag="qin")
        nc.sync.dma_start(qin[:, 0], qT_r[:, 0])
        nc.sync.dma_start(qin[:, 1], qT_r[:, 1])
        bvrow = const.tile([1, VD], F32, tag="bvrow")
        nc.scalar.dma_start(bvrow[:], aps["bv"][None, :])
        bqT = const.tile([P, NCC], F32, tag="bqT")
        nc.scalar.dma_start(bqT[:], aps["bqT"][:])
        ident = const.tile([P, P], F16, tag="ident")
        nc.scalar.dma_start(ident[:], aps["ident"][:])
        ones1 = const.tile([1, P], F32, tag="ones1")
        nc.vector.memset(ones1[:], 1.0)

        # ---- own-group K^T / V projection into the packed slab ----
        # slab[:, 0, 2*ci+h, :] = K^T chunk (ci, h)  ([c-part, keys])
        # slab[:, 1, kc, :]     = V chunk kc         ([key-part, v])
        # Each chunk is streamed to the DRAM collective-input buffer as soon
        # as its PSUM eviction lands, so the AllGather fires right after the
        # last chunk instead of waiting for a bulk 2MB copy.
        slab_own = kvown.tile([P, 2, ND, 512], F16, tag="own")
        ccin = dram.tile([P, 2, ND, 512], F16)
        for ci in range(NCC):
            for h in range(2):
                ps = pp.tile([P, 512], F32, tag="pp", name="psk")
                for d in range(ND):
                    nc.tensor.matmul(
                        ps[:],
                        wk[:, d, ci * P:(ci + 1) * P],
                        kto[:, d, h * 512:(h + 1) * 512],
                        start=(d == 0),
                        stop=(d == ND - 1),
                    )
                nc.scalar.copy(slab_own[:, 0, 2 * ci + h, :], ps[:])
                nc.gpsimd.dma_start(
                    ccin[:, 0, 2 * ci + h, :], slab_own[:, 0, 2 * ci + h, :]
                )
        for kc in range(ND):
            ps = pp.tile([P, 512], F32, tag="pp", name="psv")
            for d in range(ND):
                nc.tensor.matmul(
                    ps[:],
                    vto[:, d, kc * P:(kc + 1) * P],
                    wv[:, d, :],
                    start=(d == 0),
                    stop=(d == ND - 1),
                )
            nc.scalar.copy(slab_own[:, 1, kc, :], ps[:])
            nc.gpsimd.dma_start(ccin[:, 1, kc, :], slab_own[:, 1, kc, :])

        # ---- AllGather all groups into shared DRAM ----
        nc.gpsimd.collective_compute(
            "AllGather",
            ALU.bypass,
            replica_groups=[list(range(NCORES))],
            ins=[ccin.opt()],
            outs=[ccout.ap()],
        )

        pid = nc.sync.partition_id()
        pid_sc = nc.scalar.partition_id()
        slabs = {0: slab_own}

        pid_gp = nc.gpsimd.partition_id()
        _engs = [(nc.sync, pid), (nc.scalar, pid_sc), (nc.gpsimd, pid_gp)]

        def load_slab(j):
            # Split each slab load across the three DMA queues, and rotate
            # which queue carries the K^T half (it gates QK of the group):
            # this way the first post-collective K^T loads don't serialize
            # behind each other on one queue.
            ek, pk = _engs[(j - 1) % 3]
            ev0, pv0 = _engs[j % 3]
            ev1, pv1 = _engs[(j + 1) % 3]
            t = kvin.tile([P, 2, ND, 512], F16, tag="slab", name=f"slab{j}")
            ek.dma_start(t[:, 0], ccout[(pk + j) % NG][:, 0])
            ev0.dma_start(t[:, 1, 0:4], ccout[(pv0 + j) % NG][:, 1, 0:4])
            ev1.dma_start(t[:, 1, 4:8], ccout[(pv1 + j) % NG][:, 1, 4:8])
            slabs[j] = t

        # eager: queue the first three remote slabs on three different DMA
        # queues; they all gate on the collective's completion semaphore.
        load_slab(1)
        load_slab(2)
        load_slab(3)

        # ---- Q^T projection (rh=0 now; rh=1 chunks fill early slots) ----
        QT = const.tile([P, NCC, RL], F16, tag="QT")

        def q_chunk(rh, ci):
            ps = pp.tile([P, 512], F32, tag="pp", name="psq")
            for d in range(ND):
                nc.tensor.matmul(
                    ps[:],
                    wq[:, d, ci * P:(ci + 1) * P],
                    qin[:, rh, d, :],
                    start=(d == 0),
                    stop=(d == ND - 1),
                )
            nc.scalar.activation(
                QT[:, ci, rh * 512:(rh + 1) * 512],
                ps[:],
                AF.Identity,
                bias=bqT[:, ci:ci + 1],
                scale=1.0,
            )

        for ci in range(NCC):
            q_chunk(0, ci)

        # bvN[p, v] = NORM * bv[v] broadcast along partitions (rank-1 matmul)
        bvN = const.tile([P, VD], F32, tag="bvN")
        psb0 = pp.tile([P, VD], F32, tag="pp")
        nc.tensor.matmul(psb0[:], ones1[:], bvrow[:], start=True, stop=True)
        nc.scalar.activation(bvN[:], psb0[:], AF.Copy, bias=0.0, scale=NORM)

        # flash state, ping-pong by group-slot parity
        m_st = [const.tile([P, NB], F32, tag=f"m{i}", name=f"m{i}") for i in range(2)]
        rs_st = [const.tile([P, NB], F32, tag=f"rs{i}", name=f"rs{i}") for i in range(2)]
        O_st = [
            const.tile([P, NB, VD], F32, tag=f"O{i}", name=f"O{i}") for i in range(2)
        ]

        a_h = {}
        at_h = {}
        f_h = {}

        def qk(j, b):
            KT = slabs[j]
            m_in, m_out = m_st[j % 2], m_st[1 - j % 2]
            rs_in, rs_out = rs_st[j % 2], rs_st[1 - j % 2]
            S0 = psS.tile([P, 512], F32, tag="S0", name=f"S0_{j}_{b}")
            S1 = psS.tile([P, 512], F32, tag="S1", name=f"S1_{j}_{b}")
            for S, h in ((S0, 0), (S1, 1)):
                for ci in range(NCC):
                    nc.tensor.matmul(
                        S[:],
                        QT[:, ci, b * P:(b + 1) * P],
                        KT[:, 0, 2 * ci + h, :],
                        start=(ci == 0),
                        stop=(ci == NCC - 1),
                    )
            gm0 = stat.tile([P, 1], F32, tag="gm0", name="gm0")
            gm1 = stat.tile([P, 1], F32, tag="gm1", name="gm1")
            nc.vector.reduce_max(gm0[:], S0[:], axis=AX.X)
            nc.vector.reduce_max(gm1[:], S1[:], axis=AX.X)
            mb = m_out[:, b:b + 1]
            if j == 0:
                nc.vector.tensor_tensor(mb, gm0[:], gm1[:], op=ALU.max)
            else:
                g01 = stat.tile([P, 1], F32, tag="g01", name="g01")
                nc.vector.tensor_tensor(g01[:], gm0[:], gm1[:], op=ALU.max)
                nc.vector.tensor_tensor(mb, m_in[:, b:b + 1], g01[:], op=ALU.max)
            negm = stat.tile([P, 1], F32, tag="negm", name="negm")
            nc.vector.tensor_scalar(negm[:], mb, -1.0, None, op0=ALU.mult)
            if j > 0:
                f = stat.tile([P, 1], F32, tag="f", name="f")
                nc.scalar.activation(
                    f[:], m_in[:, b:b + 1], AF.Exp, bias=negm[:], scale=1.0
                )
                f_h[(j, b)] = f
            A = apool.tile([P, GK], F16, tag="A", name=f"A_{j}_{b}")
            ps0 = stat.tile([P, 1], F32, tag="ps0", name="ps0")
            ps1 = stat.tile([P, 1], F32, tag="ps1", name="ps1")
            nc.scalar.activation(
                A[:, 0:512], S0[:], AF.Exp, bias=negm[:], scale=1.0,
                accum_out=ps0[:],
            )
            nc.scalar.activation(
                A[:, 512:1024], S1[:], AF.Exp, bias=negm[:], scale=1.0,
                accum_out=ps1[:],
            )
            rb_in, rb_out = rs_in[:, b:b + 1], rs_out[:, b:b + 1]
            if j == 0:
                nc.vector.tensor_tensor(rb_out, ps0[:], ps1[:], op=ALU.add)
            else:
                pss = stat.tile([P, 1], F32, tag="pss", name="pss")
                nc.vector.tensor_tensor(pss[:], ps0[:], ps1[:], op=ALU.add)
                nc.vector.scalar_tensor_tensor(
                    rb_out, rb_in, f_h[(j, b)][:], pss[:], op0=ALU.mult, op1=ALU.add
                )
            a_h[(j, b)] = A

        def tr(j, b):
            # A^T for all 8 key chunks, PE transpose-mode -> one PSUM bank
            A = a_h.pop((j, b))
            tp = pat.tile([P, ND, P], F16, tag="tp", name=f"tp_{j}_{b}")
            for kc in range(ND):
                nc.tensor.transpose(
                    tp[:, kc, :], A[:, kc * P:(kc + 1) * P], ident[:]
                )
            AT = atp.tile([P, ND, P], F16, tag="AT", name=f"AT_{j}_{b}")
            nc.scalar.copy(AT[:], tp[:])
            at_h[(j, b)] = AT

        def pv(j, b):
            Vg = slabs[j]
            O_in, O_out = O_st[j % 2], O_st[1 - j % 2]
            po = pp.tile([P, VD], F32, tag="pp", name=f"po_{j}_{b}")
            AT = at_h.pop((j, b))
            for kc in range(ND):
                nc.tensor.matmul(
                    po[:],
                    AT[:, kc, :],
                    Vg[:, 1, kc, :],
                    start=(kc == 0),
                    stop=(kc == ND - 1),
                )
            ob_out, ob_in = O_out[:, b, :], O_in[:, b, :]
            if j == 0:
                nc.vector.tensor_copy(ob_out, po[:])
            else:
                nc.vector.scalar_tensor_tensor(
                    ob_out, ob_in, f_h.pop((j, b))[:], po[:],
                    op0=ALU.mult, op1=ALU.add,
                )

        def ep(j, b):
            # epilogue: out = O * (NORM / rs) + NORM * bv
            rs_out = rs_st[1 - j % 2]
            O_out = O_st[1 - j % 2]
            rinv = stat.tile([P, 1], F32, tag="rinv", name="rinv")
            nc.vector.reciprocal(rinv[:], rs_out[:, b:b + 1])
            rn = stat.tile([P, 1], F32, tag="rn", name="rn")
            nc.vector.tensor_scalar(rn[:], rinv[:], NORM, None, op0=ALU.mult)
            of = outp.tile([P, VD], F32, tag="of", name=f"of{b}")
            nc.vector.scalar_tensor_tensor(
                of[:], O_out[:, b, :], rn[:], bvN[:], op0=ALU.mult, op1=ALU.add
            )
            nc.scalar.dma_start(out_ap[b * P:(b + 1) * P, :], of[:])

        def pv_ep(j, b):
            pv(j, b)
            if j == NG - 1:
                ep(j, b)

        # early-slot PE filler: the rh=1 Q-proj chunks (needed from b=4 on)
        fillers = [
            lambda: (q_chunk(1, 0), q_chunk(1, 1)),
            lambda: q_chunk(1, 2),
            lambda: q_chunk(1, 3),
        ]

        NS = NG * NB
        for s in range(NS + 3):
            if 2 <= s < NS + 2:
                tr(*divmod(s - 2, NB))
            if s < NS:
                j, b = divmod(s, NB)
                qk(j, b)
                if s < 3:
                    fillers[s]()
                if b == 3 and j >= 1 and j + 3 < NG:
                    load_slab(j + 3)
            if s >= 3:
                pv_ep(*divmod(s - 3, NB))


_cached = {}


def _build():
    if "nc" in _cached:
        return _cached["nc"]
    nc = bass.Bass("TRN2", target_bir_lowering=False, debug=False,
                   num_devices=NCORES)
    ccout = nc.dram_tensor(
        "ccout", [NG, P, 2, ND, 512], F16, kind="Internal", addr_space="Shared"
    )
    aps = {
        "qT": nc.dram_tensor("qT", [P, 2, ND, 512], F16, kind="ExternalInput").ap(),
        "kTo": nc.dram_tensor("kTo", [P, ND, GK], F16, kind="ExternalInput").ap(),
        "vTo": nc.dram_tensor("vTo", [P, ND, GK], F16, kind="ExternalInput").ap(),
        "wqT": nc.dram_tensor("wqT", [P, ND, C], F16, kind="ExternalInput").ap(),
        "wkT": nc.dram_tensor("wkT", [P, ND, C], F16, kind="ExternalInput").ap(),
        "wvT": nc.dram_tensor("wvT", [P, ND, C], F16, kind="ExternalInput").ap(),
        "bqT": nc.dram_tensor("bqT", [P, NCC], F32, kind="ExternalInput").ap(),
        "bv": nc.dram_tensor("bv", [VD], F32, kind="ExternalInput").ap(),
        "ident": nc.dram_tensor("ident", [P, P], F16, kind="ExternalInput").ap(),
        "out": nc.dram_tensor("out", [RL, VD], F32, kind="ExternalOutput").ap(),
        "ccout": ccout,
    }
    with tile.TileContext(nc) as tc:
        _emit(nc, tc, aps)
    _split_multi_waits(nc)
    _cached["nc"] = nc
    return nc


def kernel(q, k, v, Wq, bq, Wk, bk, Wv, bv, _trace=False, _tmpdir=None):
    del bk  # provably cancels inside the softmax
    nc = _build()

    def f16(a):
        return np.asarray(a, dtype=np.float32).astype(np.float16)

    def kv_prep(a):
        # [keys, d] -> [p, group, d-chunk, key-in-group]
        return f16(a).reshape(NG, GK, ND, P).transpose(3, 0, 2, 1)

    def w_prep(a):
        # [c, d] -> [p, d-chunk, c]
        return np.ascontiguousarray(f16(a).reshape(C, ND, P).transpose(2, 1, 0))

    q16 = f16(q)
    kT = kv_prep(k)
    vT = kv_prep(v)
    base = {
        "wqT": w_prep(Wq), "wkT": w_prep(Wk), "wvT": w_prep(Wv),
        "bqT": np.ascontiguousarray(
            np.asarray(bq, dtype=np.float32).reshape(NCC, P).T
        ),
        "bv": np.ascontiguousarray(np.asarray(bv, dtype=np.float32)),
        "ident": np.eye(P, dtype=np.float16),
    }
    in_maps = [
        dict(
            base,
            qT=np.ascontiguousarray(
                q16[c * RL:(c + 1) * RL].reshape(2, 512, ND, P).transpose(3, 0, 2, 1)
            ),
            kTo=np.ascontiguousarray(kT[:, c]),
            vTo=np.ascontiguousarray(vT[:, c]),
        )
        for c in range(NCORES)
    ]
    res = bass_utils.run_bass_kernel_spmd(
        nc, in_maps, core_ids=list(range(NCORES)), trace=_trace, tmpdir=_tmpdir
    )
    out = np.concatenate([res.results[c]["out"] for c in range(NCORES)], axis=0)
    if _trace:
        kernel.last_results = res
    return out
